# revision 6
# baseline (speedup 1.0000x reference)
"""Trainium2 Bass kernel for cross "efficient attention" — v3.

Reference (per batch b, head h; C=128, HEADS=8, hc=16, n=16384):
    k = x2[b].reshape(HEADS, hc, n); v = x1[b].reshape(HEADS, hc, n)
    key_sm   = softmax(k, axis=-1)         # over n
    query_sm = softmax(k, axis=1)          # over hc
    context  = key_sm @ v^T                # (hc, hc)
    out[b,h] = context^T @ query_sm        # (hc, n)

Data-parallel over batch B=8 across 8 cores.  exp never overflows
(inputs ~N(0,1)) so softmax skips the max subtraction.

v3 structure:
 - exp_nat = exp(x2) on ScalarE, accum_out = rowsums (free).
 - Transpose-mode PE matmuls (bf16 PSUM, 8 chunks per bank) -> 1 KiB-wide
   PSUM->SBUF copies, alternating Scalar/Vector.
 - Per chunk, an 8-column sel8 matmul (same stationary) accumulates the
   per-head colsums^T into a shared f32 PSUM bank (64 chunks/bank);
   reciprocal_approx_fast reads the bank straight from PSUM.
 - Context accumulates from eT/vT (vT streamed pre-permuted bf16).
 - Tail per 16-chunk tile: transposed attended (stationary exp chunk,
   moving ctxw), DVE/GpSimd broadcast-normalize against recipT
   (stride-0 AP over the 16 channels of a head), tile-major store;
   host un-permutes.

HBM: 4 MiB x2 + 4 MiB x1 + 4 MiB out, all bf16 HWDGE multi-KiB lines.
"""

import numpy as np
from contextlib import ExitStack

B, C, H, W = 8, 128, 128, 128
N = H * W                 # 16384
HEADS, HC = 8, 16
NCORES = 8
NCH = N // C              # 128 chunks

SLABS = [2048, 4096, 4096, 2048, 2048, 1024, 512, 512]
NSLAB = len(SLABS)
assert sum(SLABS) == N
GRP = 1024                # transpose group (one bf16 PSUM bank)
CSG = 64                  # chunks per colsum PSUM bank (64*8=512 fp32)
TG = 16                   # tail chunks per PSUM tile
GPS_TILES = (1, 4, 6)     # tail tiles staged to SBUF (Scalar) for GpSimd

_cache: dict = {}


def _build():
    import concourse.bass as bass
    import concourse.tile as tile
    from concourse import bacc, mybir

    FP32 = mybir.dt.float32
    BF16 = mybir.dt.bfloat16
    AF = mybir.ActivationFunctionType

    nc = bacc.Bacc("TRN2", target_bir_lowering=False, debug=False)

    x1r = nc.dram_tensor("x1r", [C, N], BF16, kind="ExternalInput")
    x2 = nc.dram_tensor("x2", [C, N], BF16, kind="ExternalInput")
    ident_in = nc.dram_tensor("ident", [C, C], BF16, kind="ExternalInput")
    sel8_in = nc.dram_tensor("sel8", [C, HEADS], BF16, kind="ExternalInput")
    bd8_in = nc.dram_tensor("bd8", [C, C], BF16, kind="ExternalInput")
    # out[n0, j*C + v] = attended[v, j*128 + n0]  (host un-permutes)
    out = nc.dram_tensor("out", [C, N], BF16, kind="ExternalOutput")

    with tile.TileContext(nc) as tc:
        with ExitStack() as ctx:
            persist = ctx.enter_context(tc.tile_pool(name="persist", bufs=1))
            x2ld = ctx.enter_context(tc.tile_pool(name="x2ld", bufs=4))
            vTp = ctx.enter_context(tc.tile_pool(name="vTp", bufs=4))
            outp = ctx.enter_context(tc.tile_pool(name="outp", bufs=4))
            smalls = ctx.enter_context(tc.tile_pool(name="smalls", bufs=1))

            exp_nat = persist.tile([C, N], BF16, tag="exp_nat")
            eT = persist.tile([C, N], BF16, tag="eT")
            # recipT[n0, j*8 + h] = 1 / colsum[h, j*128 + n0]
            recipT = persist.tile([C, NCH * HEADS], FP32, tag="recipT")
            rs_acc = smalls.tile([C, NSLAB], FP32, tag="rs_acc")
            ident = smalls.tile([C, C], BF16, tag="ident")
            sel8 = smalls.tile([C, HEADS], BF16, tag="sel8")
            bd8 = smalls.tile([C, C], BF16, tag="bd8")

            with tc.tile_pool(name="psctx", bufs=1, space="PSUM") as ps_ctx, \
                 tc.tile_pool(name="pste", bufs=3, space="PSUM") as ps_te, \
                 tc.tile_pool(name="pscs", bufs=2, space="PSUM") as ps_cs:
                ctx_ps = ps_ctx.tile([C, C], FP32, tag="ctx")

                off = 0
                mm_idx = 0
                copy_idx = 0
                pending = None
                cs_ps = None

                def emit_ctx(eT_, vT, nch, coff):
                    nonlocal mm_idx
                    for j in range(nch):
                        nc.tensor.matmul(
                            ctx_ps[:],
                            eT_[:, bass.ds(coff + j * C, C)],  # (n0, c_k)
                            vT[:, bass.ts(j, C)],              # (n0, c_v)
                            start=(mm_idx == 0),
                            stop=(mm_idx == NCH - 1),
                        )
                        mm_idx += 1

                for i, SW in enumerate(SLABS):
                    nch = SW // C
                    sl = bass.ds(off, SW)
                    x2t = x2ld.tile([C, SW], BF16, tag="x2t")
                    nc.sync.dma_start(out=x2t[:], in_=x2[:, sl])
                    vT = vTp.tile([C, SW], BF16, tag="vT")
                    nc.gpsimd.dma_start(out=vT[:], in_=x1r[:, sl])
                    if i == 0:
                        nc.sync.dma_start(out=ident[:], in_=ident_in[:])
                        nc.sync.dma_start(out=sel8[:], in_=sel8_in[:])
                        nc.sync.dma_start(out=bd8[:], in_=bd8_in[:])

                    nc.scalar.activation(
                        exp_nat[:, sl], x2t[:], AF.Exp,
                        accum_out=rs_acc[:, i:i + 1],
                    )

                    # transpose groups + colsum matmuls
                    for g0 in range(0, SW, GRP):
                        gw = min(GRP, SW - g0)
                        te = ps_te.tile([C, gw], BF16, tag="te")
                        for j in range(gw // C):
                            jj = (off + g0) // C + j       # global chunk
                            ech = exp_nat[:, bass.ds(off + g0 + j * C, C)]
                            nc.tensor.transpose(
                                te[:, bass.ts(j, C)], ech, ident[:],
                            )
                            if jj % CSG == 0:
                                cs_ps = ps_cs.tile([C, CSG * HEADS], FP32,
                                                   tag="cs")
                            nc.tensor.matmul(
                                cs_ps[:, bass.ds((jj % CSG) * HEADS, HEADS)],
                                ech, sel8[:], start=True, stop=True,
                            )
                            if jj % CSG == CSG - 1:
                                nc.vector.reciprocal_approx_fast(
                                    out=recipT[:, bass.ds(
                                        (jj - CSG + 1) * HEADS, CSG * HEADS)],
                                    in_=cs_ps[:],
                                )
                        nc.vector.tensor_copy(eT[:, bass.ds(off + g0, gw)], te[:])

                    if pending is not None:
                        emit_ctx(*pending)
                    pending = (eT, vT, nch, off)
                    off += SW
                emit_ctx(*pending)

                # ---- context weights: scale rows by 1/rowsum, mask ----
                rowsum = smalls.tile([C, 1], FP32, tag="rowsum")
                nc.vector.tensor_reduce(
                    rowsum[:], rs_acc[:], mybir.AxisListType.X, mybir.AluOpType.add
                )
                rs_rcp = smalls.tile([C, 1], FP32, tag="rs_rcp")
                nc.vector.reciprocal(rs_rcp[:], rowsum[:])
                scaled = smalls.tile([C, C], BF16, tag="scaled")
                nc.vector.tensor_scalar(
                    scaled[:], ctx_ps[:], rs_rcp[:, 0:1], None, mybir.AluOpType.mult
                )
                bd = smalls.tile([C, C], BF16, tag="bd")
                nc.vector.tensor_mul(bd[:], scaled[:], bd8[:])

            # ---- Tail: transposed attended, broadcast normalize, store ----
            with tc.tile_pool(name="psatt", bufs=2, space="PSUM") as ps_att:
                for t in range(NCH // TG):
                    att = ps_att.tile([C, TG * C], FP32, tag="att")
                    for q in range(TG):
                        j = t * TG + q
                        nc.tensor.matmul(
                            att[:, bass.ts(q, C)],
                            exp_nat[:, bass.ts(j, C)],   # stationary (k, n0)
                            bd[:],                        # moving (k, v)
                            start=True, stop=True,
                        )
                    ot = outp.tile([C, TG * C], BF16, tag="ot")
                    r_view = recipT[:, bass.ds(t * TG * HEADS, TG * HEADS)] \
                        .rearrange("p (q h) -> p q h", q=TG) \
                        .unsqueeze(3).broadcast_to([C, TG, HEADS, HC])
                    o_view = ot[:].rearrange(
                        "p (q h i) -> p q h i", q=TG, h=HEADS
                    )
                    if t in GPS_TILES:
                        # DVE is the tail bottleneck; GpSimd can't read PSUM,
                        # so Scalar (idle here) stages att to SBUF bf16 first.
                        stg = outp.tile([C, TG * C], BF16, tag="stg")
                        nc.scalar.copy(stg[:], att[:])
                        s_view = stg[:].rearrange(
                            "p (q h i) -> p q h i", q=TG, h=HEADS
                        )
                        nc.gpsimd.tensor_tensor(
                            o_view, s_view, r_view, mybir.AluOpType.mult
                        )
                    else:
                        a_view = att[:].rearrange(
                            "p (q h i) -> p q h i", q=TG, h=HEADS
                        )
                        nc.vector.tensor_tensor(
                            o_view, a_view, r_view, mybir.AluOpType.mult
                        )
                    nc.sync.dma_start(
                        out=out[:, bass.ds(t * TG * C, TG * C)], in_=ot[:]
                    )

    nc.compile()
    return nc


def _get_nc():
    if "nc" not in _cache:
        _cache["nc"] = _build()
    return _cache["nc"]


def _ident_np() -> np.ndarray:
    import ml_dtypes

    return np.eye(C, dtype=np.float32).astype(ml_dtypes.bfloat16)


def _sel8_np() -> np.ndarray:
    import ml_dtypes

    m = np.zeros((C, HEADS), dtype=np.float32)
    for h in range(HEADS):
        m[h * HC:(h + 1) * HC, h] = 1.0
    return m.astype(ml_dtypes.bfloat16)


def _bd8_np() -> np.ndarray:
    import ml_dtypes

    m = np.zeros((C, C), dtype=np.float32)
    for h in range(HEADS):
        m[h * HC:(h + 1) * HC, h * HC:(h + 1) * HC] = 1.0
    return m.astype(ml_dtypes.bfloat16)


def _to_np(a) -> np.ndarray:
    out = np.asarray(a, dtype=np.float32)
    if np.isnan(out).any():
        out = np.asarray(a, dtype=np.float32)
    return out


def _make_in_maps(x1: np.ndarray, x2: np.ndarray) -> list:
    """x1, x2: (B, C, N) float32.  Returns per-core input dicts."""
    import ml_dtypes

    BF = ml_dtypes.bfloat16
    # x1r[p, j*C + c] = x1[b, c, j*128 + p]
    x1r = np.ascontiguousarray(
        x1.reshape(B, C, N // 128, 128).transpose(0, 3, 2, 1)
    ).reshape(B, 128, N).astype(BF)
    x2b = np.ascontiguousarray(x2).astype(BF)
    ident = _ident_np()
    sel8 = _sel8_np()
    bd8 = _bd8_np()
    return [
        {"x1r": x1r[i], "x2": x2b[i], "ident": ident, "sel8": sel8, "bd8": bd8}
        for i in range(NCORES)
    ]


def _unpermute_out(raw: np.ndarray) -> np.ndarray:
    """raw: (C, N) with raw[n0, j*C + v] = att[v, j*128 + n0]."""
    return np.ascontiguousarray(
        raw.reshape(128, NCH, C).transpose(2, 1, 0)
    ).reshape(C, N)


def kernel(x1: np.ndarray, x2: np.ndarray) -> np.ndarray:
    from concourse.bass_utils import run_bass_kernel_spmd

    nc = _get_nc()
    x1 = _to_np(x1).reshape(B, C, N)
    x2 = _to_np(x2).reshape(B, C, N)
    in_maps = _make_in_maps(x1, x2)
    res = run_bass_kernel_spmd(nc, in_maps, core_ids=list(range(NCORES)))
    outs = [
        _unpermute_out(np.asarray(res.results[i]["out"], dtype=np.float32))
        for i in range(NCORES)
    ]
    return np.stack(outs, axis=0).reshape(B, C, H, W)


# revision 7
# speedup vs baseline: 1.1574x; 1.1574x over previous
"""Trainium2 Bass kernel for cross "efficient attention" — v3.

Reference (per batch b, head h; C=128, HEADS=8, hc=16, n=16384):
    k = x2[b].reshape(HEADS, hc, n); v = x1[b].reshape(HEADS, hc, n)
    key_sm   = softmax(k, axis=-1)         # over n
    query_sm = softmax(k, axis=1)          # over hc
    context  = key_sm @ v^T                # (hc, hc)
    out[b,h] = context^T @ query_sm        # (hc, n)

Data-parallel over batch B=8 across 8 cores.  exp never overflows
(inputs ~N(0,1)) so softmax skips the max subtraction.

v3 structure:
 - exp_nat = exp(x2) on ScalarE, accum_out = rowsums (free).
 - Transpose-mode PE matmuls (bf16 PSUM, 8 chunks per bank) -> 1 KiB-wide
   PSUM->SBUF copies, alternating Scalar/Vector.
 - Per chunk, an 8-column sel8 matmul (same stationary) accumulates the
   per-head colsums^T into a shared f32 PSUM bank (64 chunks/bank);
   reciprocal_approx_fast reads the bank straight from PSUM.
 - Context accumulates from eT/vT (vT streamed pre-permuted bf16).
 - Tail per 16-chunk tile: transposed attended (stationary exp chunk,
   moving ctxw), DVE/GpSimd broadcast-normalize against recipT
   (stride-0 AP over the 16 channels of a head), tile-major store;
   host un-permutes.

HBM: 4 MiB x2 + 4 MiB x1 + 4 MiB out, all bf16 HWDGE multi-KiB lines.
"""

import numpy as np
from contextlib import ExitStack

B, C, H, W = 8, 128, 128, 128
N = H * W                 # 16384
HEADS, HC = 8, 16
NCORES = 8
NCH = N // C              # 128 chunks

SLABS = [2048, 4096, 4096, 2048, 2048, 1024, 512, 512]
NSLAB = len(SLABS)
assert sum(SLABS) == N
GRP = 1024                # transpose group (one bf16 PSUM bank)
CSG = 64                  # chunks per colsum PSUM bank (64*8=512 fp32)
TG = 16                   # tail chunks per PSUM tile
GPS_TILES = ()            # GpSimd TT measured ~3x slower than modeled; all-DVE tail

_cache: dict = {}


def _build():
    import concourse.bass as bass
    import concourse.tile as tile
    from concourse import bacc, mybir

    FP32 = mybir.dt.float32
    BF16 = mybir.dt.bfloat16
    AF = mybir.ActivationFunctionType

    nc = bacc.Bacc("TRN2", target_bir_lowering=False, debug=False)

    x1r = nc.dram_tensor("x1r", [C, N], BF16, kind="ExternalInput")
    x2 = nc.dram_tensor("x2", [C, N], BF16, kind="ExternalInput")
    ident_in = nc.dram_tensor("ident", [C, C], BF16, kind="ExternalInput")
    sel8_in = nc.dram_tensor("sel8", [C, HEADS], BF16, kind="ExternalInput")
    bd8_in = nc.dram_tensor("bd8", [C, C], BF16, kind="ExternalInput")
    # out[n0, j*C + v] = attended[v, j*128 + n0]  (host un-permutes)
    out = nc.dram_tensor("out", [C, N], BF16, kind="ExternalOutput")

    with tile.TileContext(nc) as tc:
        with ExitStack() as ctx:
            persist = ctx.enter_context(tc.tile_pool(name="persist", bufs=1))
            x2ld = ctx.enter_context(tc.tile_pool(name="x2ld", bufs=4))
            vTp = ctx.enter_context(tc.tile_pool(name="vTp", bufs=4))
            outp = ctx.enter_context(tc.tile_pool(name="outp", bufs=4))
            smalls = ctx.enter_context(tc.tile_pool(name="smalls", bufs=1))

            exp_nat = persist.tile([C, N], BF16, tag="exp_nat")
            eT = persist.tile([C, N], BF16, tag="eT")
            # recipT[n0, j*8 + h] = 1 / colsum[h, j*128 + n0]
            recipT = persist.tile([C, NCH * HEADS], FP32, tag="recipT")
            rs_acc = smalls.tile([C, NSLAB], FP32, tag="rs_acc")
            ident = smalls.tile([C, C], BF16, tag="ident")
            sel8 = smalls.tile([C, HEADS], BF16, tag="sel8")
            bd8 = smalls.tile([C, C], BF16, tag="bd8")

            with tc.tile_pool(name="psctx", bufs=1, space="PSUM") as ps_ctx, \
                 tc.tile_pool(name="pste", bufs=3, space="PSUM") as ps_te, \
                 tc.tile_pool(name="pscs", bufs=2, space="PSUM") as ps_cs:
                ctx_ps = ps_ctx.tile([C, C], FP32, tag="ctx")

                off = 0
                mm_idx = 0
                copy_idx = 0
                pending = None
                cs_ps = None

                def emit_ctx(eT_, vT, nch, coff):
                    nonlocal mm_idx
                    for j in range(nch):
                        nc.tensor.matmul(
                            ctx_ps[:],
                            eT_[:, bass.ds(coff + j * C, C)],  # (n0, c_k)
                            vT[:, bass.ts(j, C)],              # (n0, c_v)
                            start=(mm_idx == 0),
                            stop=(mm_idx == NCH - 1),
                        )
                        mm_idx += 1

                for i, SW in enumerate(SLABS):
                    nch = SW // C
                    sl = bass.ds(off, SW)
                    x2t = x2ld.tile([C, SW], BF16, tag="x2t")
                    nc.sync.dma_start(out=x2t[:], in_=x2[:, sl])
                    vT = vTp.tile([C, SW], BF16, tag="vT")
                    nc.gpsimd.dma_start(out=vT[:], in_=x1r[:, sl])
                    if i == 0:
                        nc.sync.dma_start(out=ident[:], in_=ident_in[:])
                        nc.sync.dma_start(out=sel8[:], in_=sel8_in[:])
                        nc.sync.dma_start(out=bd8[:], in_=bd8_in[:])

                    nc.scalar.activation(
                        exp_nat[:, sl], x2t[:], AF.Exp,
                        accum_out=rs_acc[:, i:i + 1],
                    )

                    # transpose groups + colsum matmuls
                    for g0 in range(0, SW, GRP):
                        gw = min(GRP, SW - g0)
                        te = ps_te.tile([C, gw], BF16, tag="te")
                        for j in range(gw // C):
                            jj = (off + g0) // C + j       # global chunk
                            ech = exp_nat[:, bass.ds(off + g0 + j * C, C)]
                            nc.tensor.transpose(
                                te[:, bass.ts(j, C)], ech, ident[:],
                            )
                            if jj % CSG == 0:
                                cs_ps = ps_cs.tile([C, CSG * HEADS], FP32,
                                                   tag="cs")
                            nc.tensor.matmul(
                                cs_ps[:, bass.ds((jj % CSG) * HEADS, HEADS)],
                                ech, sel8[:], start=True, stop=True,
                            )
                            if jj % CSG == CSG - 1:
                                nc.vector.reciprocal_approx_fast(
                                    out=recipT[:, bass.ds(
                                        (jj - CSG + 1) * HEADS, CSG * HEADS)],
                                    in_=cs_ps[:],
                                )
                        nc.vector.tensor_copy(eT[:, bass.ds(off + g0, gw)], te[:])

                    if pending is not None:
                        emit_ctx(*pending)
                    pending = (eT, vT, nch, off)
                    off += SW
                emit_ctx(*pending)

                # ---- context weights: scale rows by 1/rowsum, mask ----
                rowsum = smalls.tile([C, 1], FP32, tag="rowsum")
                nc.vector.tensor_reduce(
                    rowsum[:], rs_acc[:], mybir.AxisListType.X, mybir.AluOpType.add
                )
                rs_rcp = smalls.tile([C, 1], FP32, tag="rs_rcp")
                nc.vector.reciprocal(rs_rcp[:], rowsum[:])
                scaled = smalls.tile([C, C], BF16, tag="scaled")
                nc.vector.tensor_scalar(
                    scaled[:], ctx_ps[:], rs_rcp[:, 0:1], None, mybir.AluOpType.mult
                )
                bd = smalls.tile([C, C], BF16, tag="bd")
                nc.vector.tensor_mul(bd[:], scaled[:], bd8[:])

            # ---- Tail: transposed attended, broadcast normalize, store ----
            with tc.tile_pool(name="psatt", bufs=2, space="PSUM") as ps_att:
                for t in range(NCH // TG):
                    att = ps_att.tile([C, TG * C], FP32, tag="att")
                    for q in range(TG):
                        j = t * TG + q
                        nc.tensor.matmul(
                            att[:, bass.ts(q, C)],
                            exp_nat[:, bass.ts(j, C)],   # stationary (k, n0)
                            bd[:],                        # moving (k, v)
                            start=True, stop=True,
                        )
                    ot = outp.tile([C, TG * C], BF16, tag="ot")
                    r_view = recipT[:, bass.ds(t * TG * HEADS, TG * HEADS)] \
                        .rearrange("p (q h) -> p q h", q=TG) \
                        .unsqueeze(3).broadcast_to([C, TG, HEADS, HC])
                    o_view = ot[:].rearrange(
                        "p (q h i) -> p q h i", q=TG, h=HEADS
                    )
                    if t in GPS_TILES:
                        # DVE is the tail bottleneck; GpSimd can't read PSUM,
                        # so Scalar (idle here) stages att to SBUF bf16 first.
                        stg = outp.tile([C, TG * C], BF16, tag="stg")
                        nc.scalar.copy(stg[:], att[:])
                        s_view = stg[:].rearrange(
                            "p (q h i) -> p q h i", q=TG, h=HEADS
                        )
                        nc.gpsimd.tensor_tensor(
                            o_view, s_view, r_view, mybir.AluOpType.mult
                        )
                    else:
                        a_view = att[:].rearrange(
                            "p (q h i) -> p q h i", q=TG, h=HEADS
                        )
                        nc.vector.tensor_tensor(
                            o_view, a_view, r_view, mybir.AluOpType.mult
                        )
                    nc.sync.dma_start(
                        out=out[:, bass.ds(t * TG * C, TG * C)], in_=ot[:]
                    )

    nc.compile()
    return nc


def _get_nc():
    if "nc" not in _cache:
        _cache["nc"] = _build()
    return _cache["nc"]


def _ident_np() -> np.ndarray:
    import ml_dtypes

    return np.eye(C, dtype=np.float32).astype(ml_dtypes.bfloat16)


def _sel8_np() -> np.ndarray:
    import ml_dtypes

    m = np.zeros((C, HEADS), dtype=np.float32)
    for h in range(HEADS):
        m[h * HC:(h + 1) * HC, h] = 1.0
    return m.astype(ml_dtypes.bfloat16)


def _bd8_np() -> np.ndarray:
    import ml_dtypes

    m = np.zeros((C, C), dtype=np.float32)
    for h in range(HEADS):
        m[h * HC:(h + 1) * HC, h * HC:(h + 1) * HC] = 1.0
    return m.astype(ml_dtypes.bfloat16)


def _to_np(a) -> np.ndarray:
    out = np.asarray(a, dtype=np.float32)
    if np.isnan(out).any():
        out = np.asarray(a, dtype=np.float32)
    return out


def _make_in_maps(x1: np.ndarray, x2: np.ndarray) -> list:
    """x1, x2: (B, C, N) float32.  Returns per-core input dicts."""
    import ml_dtypes

    BF = ml_dtypes.bfloat16
    # x1r[p, j*C + c] = x1[b, c, j*128 + p]
    x1r = np.ascontiguousarray(
        x1.reshape(B, C, N // 128, 128).transpose(0, 3, 2, 1)
    ).reshape(B, 128, N).astype(BF)
    x2b = np.ascontiguousarray(x2).astype(BF)
    ident = _ident_np()
    sel8 = _sel8_np()
    bd8 = _bd8_np()
    return [
        {"x1r": x1r[i], "x2": x2b[i], "ident": ident, "sel8": sel8, "bd8": bd8}
        for i in range(NCORES)
    ]


def _unpermute_out(raw: np.ndarray) -> np.ndarray:
    """raw: (C, N) with raw[n0, j*C + v] = att[v, j*128 + n0]."""
    return np.ascontiguousarray(
        raw.reshape(128, NCH, C).transpose(2, 1, 0)
    ).reshape(C, N)


def kernel(x1: np.ndarray, x2: np.ndarray) -> np.ndarray:
    from concourse.bass_utils import run_bass_kernel_spmd

    nc = _get_nc()
    x1 = _to_np(x1).reshape(B, C, N)
    x2 = _to_np(x2).reshape(B, C, N)
    in_maps = _make_in_maps(x1, x2)
    res = run_bass_kernel_spmd(nc, in_maps, core_ids=list(range(NCORES)))
    outs = [
        _unpermute_out(np.asarray(res.results[i]["out"], dtype=np.float32))
        for i in range(NCORES)
    ]
    return np.stack(outs, axis=0).reshape(B, C, H, W)


# revision 8
# speedup vs baseline: 1.2689x; 1.0963x over previous
"""Trainium2 Bass kernel for cross "efficient attention" — v6.

Reference (per batch b, head h; C=128, HEADS=8, hc=16, n=16384):
    k = x2[b].reshape(HEADS, hc, n); v = x1[b].reshape(HEADS, hc, n)
    key_sm   = softmax(k, axis=-1)         # over n
    query_sm = softmax(k, axis=1)          # over hc
    context  = key_sm @ v^T                # (hc, hc)
    out[b,h] = context^T @ query_sm        # (hc, n)

Data-parallel over batch B=8 across 8 cores.  exp never overflows
(inputs ~N(0,1)) so softmax skips the max subtraction.

v6 pipeline (per 1024-column group, one group late for ctx):
 - exp_nat = exp(x2) on ScalarE, accum_out = rowsums (free).
 - Transpose-mode PE matmuls (bf16 PSUM bank per group) -> one DVE
   PSUM->SBUF copy per group.
 - Per chunk, an 8-column sel8 matmul (same stationary) accumulates
   per-head colsums^T into a shared f32 PSUM bank (64 chunks/bank);
   reciprocal_approx_fast reads it straight from PSUM.
 - ctx accumulates from eT/vT one GROUP late so the PE never
   head-of-line-blocks the vT pool recycle on the next slab's exp.
 - Tail per 16-chunk tile: transposed attended (stationary exp chunk,
   moving ctxw), DVE broadcast-normalize against recipT (stride-0 AP
   over the 16 channels of a head), tile-major store; host un-permutes.

HBM: 4 MiB x2 + 4 MiB x1 + 4 MiB out, all bf16 HWDGE multi-KiB lines.
Constants prefetch on the GpSimd queue ahead of x1r so the PE's ident
arrives before the first exp group completes.
"""

import numpy as np
from contextlib import ExitStack

B, C, H, W = 8, 128, 128, 128
N = H * W                 # 16384
HEADS, HC = 8, 16
NCORES = 8
NCH = N // C              # 128 chunks

SLABS = [1024, 1024] + [2048] * 6 + [1024, 1024]
NSLAB = len(SLABS)
assert sum(SLABS) == N
GRP = 1024                # transpose group (one bf16 PSUM bank)
CSG = 64                  # chunks per colsum PSUM bank
TG = 16                   # tail chunks per PSUM tile

_cache: dict = {}


def _build():
    import concourse.bass as bass
    import concourse.tile as tile
    from concourse import bacc, mybir

    FP32 = mybir.dt.float32
    BF16 = mybir.dt.bfloat16
    AF = mybir.ActivationFunctionType

    nc = bacc.Bacc("TRN2", target_bir_lowering=False, debug=False)

    x1r = nc.dram_tensor("x1r", [C, N], BF16, kind="ExternalInput")
    x2 = nc.dram_tensor("x2", [C, N], BF16, kind="ExternalInput")
    ident_in = nc.dram_tensor("ident", [C, C], BF16, kind="ExternalInput")
    sel8_in = nc.dram_tensor("sel8", [C, HEADS], BF16, kind="ExternalInput")
    bd8_in = nc.dram_tensor("bd8", [C, C], BF16, kind="ExternalInput")
    # out[n0, j*C + v] = attended[v, j*128 + n0]  (host un-permutes)
    out = nc.dram_tensor("out", [C, N], BF16, kind="ExternalOutput")

    with tile.TileContext(nc) as tc:
        with ExitStack() as ctx:
            persist = ctx.enter_context(tc.tile_pool(name="persist", bufs=1))
            x2ld = ctx.enter_context(tc.tile_pool(name="x2ld", bufs=6))
            vTp = ctx.enter_context(tc.tile_pool(name="vTp", bufs=6))
            outp = ctx.enter_context(tc.tile_pool(name="outp", bufs=4))
            smalls = ctx.enter_context(tc.tile_pool(name="smalls", bufs=1))

            exp_nat = persist.tile([C, N], BF16, tag="exp_nat")
            eT = persist.tile([C, N], BF16, tag="eT")
            # recipT[n0, j*8 + h] = 1 / colsum[h, j*128 + n0]
            recipT = persist.tile([C, NCH * HEADS], FP32, tag="recipT")
            rs_acc = smalls.tile([C, NSLAB], FP32, tag="rs_acc")
            ident = smalls.tile([C, C], BF16, tag="ident")
            sel8 = smalls.tile([C, HEADS], BF16, tag="sel8")
            bd8 = smalls.tile([C, C], BF16, tag="bd8")

            # constants ride the (otherwise idle this early) GpSimd queue
            nc.gpsimd.dma_start(out=ident[:], in_=ident_in[:])
            nc.gpsimd.dma_start(out=sel8[:], in_=sel8_in[:])
            nc.gpsimd.dma_start(out=bd8[:], in_=bd8_in[:])

            with tc.tile_pool(name="psctx", bufs=1, space="PSUM") as ps_ctx, \
                 tc.tile_pool(name="pste", bufs=3, space="PSUM") as ps_te, \
                 tc.tile_pool(name="pscs", bufs=2, space="PSUM") as ps_cs:
                ctx_ps = ps_ctx.tile([C, C], FP32, tag="ctx")

                mm_idx = 0
                cs_ps = None
                pending = None   # (goff, gw, vT, slab_off) one GROUP late

                def emit_ctx(goff, gw, vT, slab_off):
                    nonlocal mm_idx
                    for j in range(gw // C):
                        nc.tensor.matmul(
                            ctx_ps[:],
                            eT[:, bass.ds(goff + j * C, C)],           # (n0, ck)
                            vT[:, bass.ds(goff - slab_off + j * C, C)],  # (n0, cv)
                            start=(mm_idx == 0),
                            stop=(mm_idx == NCH - 1),
                        )
                        mm_idx += 1

                off = 0
                for i, SW in enumerate(SLABS):
                    sl = bass.ds(off, SW)
                    x2t = x2ld.tile([C, SW], BF16, tag="x2t")
                    nc.sync.dma_start(out=x2t[:], in_=x2[:, sl])
                    vT = vTp.tile([C, SW], BF16, tag="vT")
                    nc.gpsimd.dma_start(out=vT[:], in_=x1r[:, sl])

                    nc.scalar.activation(
                        exp_nat[:, sl], x2t[:], AF.Exp,
                        accum_out=rs_acc[:, i:i + 1],
                    )

                    for g0 in range(0, SW, GRP):
                        gw = min(GRP, SW - g0)
                        te = ps_te.tile([C, gw], BF16, tag="te")
                        for j in range(gw // C):
                            jj = (off + g0) // C + j       # global chunk
                            ech = exp_nat[:, bass.ds(off + g0 + j * C, C)]
                            nc.tensor.transpose(
                                te[:, bass.ts(j, C)], ech, ident[:],
                            )
                            if jj % CSG == 0:
                                cs_ps = ps_cs.tile([C, CSG * HEADS], FP32,
                                                   tag="cs")
                            nc.tensor.matmul(
                                cs_ps[:, bass.ds((jj % CSG) * HEADS, HEADS)],
                                ech, sel8[:], start=True, stop=True,
                            )
                            if jj % CSG == CSG - 1:
                                nc.vector.reciprocal_approx_fast(
                                    out=recipT[:, bass.ds(
                                        (jj - CSG + 1) * HEADS, CSG * HEADS)],
                                    in_=cs_ps[:],
                                )
                        nc.vector.tensor_copy(eT[:, bass.ds(off + g0, gw)],
                                              te[:])
                        if pending is not None:
                            emit_ctx(*pending)
                        pending = (off + g0, gw, vT, off)
                    off += SW
                emit_ctx(*pending)

                # ---- context weights: scale rows by 1/rowsum, mask ----
                rowsum = smalls.tile([C, 1], FP32, tag="rowsum")
                nc.vector.tensor_reduce(
                    rowsum[:], rs_acc[:], mybir.AxisListType.X, mybir.AluOpType.add
                )
                rs_rcp = smalls.tile([C, 1], FP32, tag="rs_rcp")
                nc.vector.reciprocal(rs_rcp[:], rowsum[:])
                scaled = smalls.tile([C, C], BF16, tag="scaled")
                nc.vector.tensor_scalar(
                    scaled[:], ctx_ps[:], rs_rcp[:, 0:1], None, mybir.AluOpType.mult
                )
                bd = smalls.tile([C, C], BF16, tag="bd")
                nc.vector.tensor_mul(bd[:], scaled[:], bd8[:])

            # ---- Tail: transposed attended, broadcast normalize, store ----
            with tc.tile_pool(name="psatt", bufs=2, space="PSUM") as ps_att:
                for t in range(NCH // TG):
                    att = ps_att.tile([C, TG * C], FP32, tag="att")
                    for q in range(TG):
                        j = t * TG + q
                        nc.tensor.matmul(
                            att[:, bass.ts(q, C)],
                            exp_nat[:, bass.ts(j, C)],   # stationary (k, n0)
                            bd[:],                        # moving (k, v)
                            start=True, stop=True,
                        )
                    ot = outp.tile([C, TG * C], BF16, tag="ot")
                    a_view = att[:].rearrange(
                        "p (q h i) -> p q h i", q=TG, h=HEADS
                    )
                    r_view = recipT[:, bass.ds(t * TG * HEADS, TG * HEADS)] \
                        .rearrange("p (q h) -> p q h", q=TG) \
                        .unsqueeze(3).broadcast_to([C, TG, HEADS, HC])
                    o_view = ot[:].rearrange(
                        "p (q h i) -> p q h i", q=TG, h=HEADS
                    )
                    nc.vector.tensor_tensor(
                        o_view, a_view, r_view, mybir.AluOpType.mult
                    )
                    nc.sync.dma_start(
                        out=out[:, bass.ds(t * TG * C, TG * C)], in_=ot[:]
                    )

    nc.compile()
    return nc


def _get_nc():
    if "nc" not in _cache:
        _cache["nc"] = _build()
    return _cache["nc"]


def _ident_np() -> np.ndarray:
    import ml_dtypes

    return np.eye(C, dtype=np.float32).astype(ml_dtypes.bfloat16)


def _sel8_np() -> np.ndarray:
    import ml_dtypes

    m = np.zeros((C, HEADS), dtype=np.float32)
    for h in range(HEADS):
        m[h * HC:(h + 1) * HC, h] = 1.0
    return m.astype(ml_dtypes.bfloat16)


def _bd8_np() -> np.ndarray:
    import ml_dtypes

    m = np.zeros((C, C), dtype=np.float32)
    for h in range(HEADS):
        m[h * HC:(h + 1) * HC, h * HC:(h + 1) * HC] = 1.0
    return m.astype(ml_dtypes.bfloat16)


def _to_np(a) -> np.ndarray:
    out = np.asarray(a, dtype=np.float32)
    if np.isnan(out).any():
        out = np.asarray(a, dtype=np.float32)
    return out


def _make_in_maps(x1: np.ndarray, x2: np.ndarray) -> list:
    """x1, x2: (B, C, N) float32.  Returns per-core input dicts."""
    import ml_dtypes

    BF = ml_dtypes.bfloat16
    # x1r[p, j*C + c] = x1[b, c, j*128 + p]
    x1r = np.ascontiguousarray(
        x1.reshape(B, C, N // 128, 128).transpose(0, 3, 2, 1)
    ).reshape(B, 128, N).astype(BF)
    x2b = np.ascontiguousarray(x2).astype(BF)
    ident = _ident_np()
    sel8 = _sel8_np()
    bd8 = _bd8_np()
    return [
        {"x1r": x1r[i], "x2": x2b[i], "ident": ident, "sel8": sel8, "bd8": bd8}
        for i in range(NCORES)
    ]


def _unpermute_out(raw: np.ndarray) -> np.ndarray:
    """raw: (C, N) with raw[n0, j*C + v] = att[v, j*128 + n0]."""
    return np.ascontiguousarray(
        raw.reshape(128, NCH, C).transpose(2, 1, 0)
    ).reshape(C, N)


def kernel(x1: np.ndarray, x2: np.ndarray) -> np.ndarray:
    from concourse.bass_utils import run_bass_kernel_spmd

    nc = _get_nc()
    x1 = _to_np(x1).reshape(B, C, N)
    x2 = _to_np(x2).reshape(B, C, N)
    in_maps = _make_in_maps(x1, x2)
    res = run_bass_kernel_spmd(nc, in_maps, core_ids=list(range(NCORES)))
    outs = [
        _unpermute_out(np.asarray(res.results[i]["out"], dtype=np.float32))
        for i in range(NCORES)
    ]
    return np.stack(outs, axis=0).reshape(B, C, H, W)


# revision 9
# speedup vs baseline: 1.2960x; 1.0213x over previous
"""Trainium2 Bass kernel for cross "efficient attention" — v6.

Reference (per batch b, head h; C=128, HEADS=8, hc=16, n=16384):
    k = x2[b].reshape(HEADS, hc, n); v = x1[b].reshape(HEADS, hc, n)
    key_sm   = softmax(k, axis=-1)         # over n
    query_sm = softmax(k, axis=1)          # over hc
    context  = key_sm @ v^T                # (hc, hc)
    out[b,h] = context^T @ query_sm        # (hc, n)

Data-parallel over batch B=8 across 8 cores.  exp never overflows
(inputs ~N(0,1)) so softmax skips the max subtraction.

v6 pipeline (per 1024-column group, one group late for ctx):
 - exp_nat = exp(x2) on ScalarE, accum_out = rowsums (free).
 - Transpose-mode PE matmuls (bf16 PSUM bank per group) -> one DVE
   PSUM->SBUF copy per group.
 - Per chunk, an 8-column sel8 matmul (same stationary) accumulates
   per-head colsums^T into a shared f32 PSUM bank (64 chunks/bank);
   reciprocal_approx_fast reads it straight from PSUM.
 - ctx accumulates from eT/vT one GROUP late so the PE never
   head-of-line-blocks the vT pool recycle on the next slab's exp.
 - Tail per 16-chunk tile: transposed attended (stationary exp chunk,
   moving ctxw), DVE broadcast-normalize against recipT (stride-0 AP
   over the 16 channels of a head), tile-major store; host un-permutes.

HBM: 4 MiB x2 + 4 MiB x1 + 4 MiB out, all bf16 HWDGE multi-KiB lines.
Constants prefetch on the GpSimd queue ahead of x1r so the PE's ident
arrives before the first exp group completes.
"""

import numpy as np
from contextlib import ExitStack

B, C, H, W = 8, 128, 128, 128
N = H * W                 # 16384
HEADS, HC = 8, 16
NCORES = 8
NCH = N // C              # 128 chunks

SLABS = [1024, 1024] + [2048] * 6 + [1024, 512, 512]
NSLAB = len(SLABS)
assert sum(SLABS) == N
GRP = 1024                # transpose group (one bf16 PSUM bank)
CSG = 64                  # chunks per colsum PSUM bank
TG = 16                   # tail chunks per PSUM tile

_cache: dict = {}


def _build():
    import concourse.bass as bass
    import concourse.tile as tile
    from concourse import bacc, mybir

    FP32 = mybir.dt.float32
    BF16 = mybir.dt.bfloat16
    AF = mybir.ActivationFunctionType

    nc = bacc.Bacc("TRN2", target_bir_lowering=False, debug=False)

    x1r = nc.dram_tensor("x1r", [C, N], BF16, kind="ExternalInput")
    x2 = nc.dram_tensor("x2", [C, N], BF16, kind="ExternalInput")
    ident_in = nc.dram_tensor("ident", [C, C], BF16, kind="ExternalInput")
    sel8_in = nc.dram_tensor("sel8", [C, HEADS], BF16, kind="ExternalInput")
    bd8_in = nc.dram_tensor("bd8", [C, C], BF16, kind="ExternalInput")
    # out[n0, j*C + v] = attended[v, j*128 + n0]  (host un-permutes)
    out = nc.dram_tensor("out", [C, N], BF16, kind="ExternalOutput")

    with tile.TileContext(nc) as tc:
        with ExitStack() as ctx:
            persist = ctx.enter_context(tc.tile_pool(name="persist", bufs=1))
            x2ld = ctx.enter_context(tc.tile_pool(name="x2ld", bufs=6))
            vTp = ctx.enter_context(tc.tile_pool(name="vTp", bufs=6))
            outp = ctx.enter_context(tc.tile_pool(name="outp", bufs=4))
            smalls = ctx.enter_context(tc.tile_pool(name="smalls", bufs=1))

            exp_nat = persist.tile([C, N], BF16, tag="exp_nat")
            eT = persist.tile([C, N], BF16, tag="eT")
            # recipT[n0, j*8 + h] = 1 / colsum[h, j*128 + n0]
            recipT = persist.tile([C, NCH * HEADS], FP32, tag="recipT")
            rs_acc = smalls.tile([C, NSLAB], FP32, tag="rs_acc")
            ident = smalls.tile([C, C], BF16, tag="ident")
            sel8 = smalls.tile([C, HEADS], BF16, tag="sel8")
            bd8 = smalls.tile([C, C], BF16, tag="bd8")

            # constants ride the (otherwise idle this early) GpSimd queue
            nc.gpsimd.dma_start(out=ident[:], in_=ident_in[:])
            nc.gpsimd.dma_start(out=sel8[:], in_=sel8_in[:])
            nc.gpsimd.dma_start(out=bd8[:], in_=bd8_in[:])

            with tc.tile_pool(name="psctx", bufs=1, space="PSUM") as ps_ctx, \
                 tc.tile_pool(name="pste", bufs=3, space="PSUM") as ps_te, \
                 tc.tile_pool(name="pscs", bufs=2, space="PSUM") as ps_cs:
                ctx_ps = ps_ctx.tile([C, C], FP32, tag="ctx")

                mm_idx = 0
                cs_ps = None
                pending = None   # (goff, gw, vT, slab_off) one GROUP late

                def emit_ctx(goff, gw, vT, slab_off):
                    nonlocal mm_idx
                    for j in range(gw // C):
                        nc.tensor.matmul(
                            ctx_ps[:],
                            eT[:, bass.ds(goff + j * C, C)],           # (n0, ck)
                            vT[:, bass.ds(goff - slab_off + j * C, C)],  # (n0, cv)
                            start=(mm_idx == 0),
                            stop=(mm_idx == NCH - 1),
                        )
                        mm_idx += 1

                off = 0
                for i, SW in enumerate(SLABS):
                    sl = bass.ds(off, SW)
                    x2t = x2ld.tile([C, SW], BF16, tag="x2t")
                    nc.sync.dma_start(out=x2t[:], in_=x2[:, sl])
                    vT = vTp.tile([C, SW], BF16, tag="vT")
                    nc.scalar.dma_start(out=vT[:], in_=x1r[:, sl])

                    nc.scalar.activation(
                        exp_nat[:, sl], x2t[:], AF.Exp,
                        accum_out=rs_acc[:, i:i + 1],
                    )

                    for g0 in range(0, SW, GRP):
                        gw = min(GRP, SW - g0)
                        te = ps_te.tile([C, gw], BF16, tag="te")
                        for j in range(gw // C):
                            jj = (off + g0) // C + j       # global chunk
                            ech = exp_nat[:, bass.ds(off + g0 + j * C, C)]
                            nc.tensor.transpose(
                                te[:, bass.ts(j, C)], ech, ident[:],
                            )
                            if jj % CSG == 0:
                                cs_ps = ps_cs.tile([C, CSG * HEADS], FP32,
                                                   tag="cs")
                            nc.tensor.matmul(
                                cs_ps[:, bass.ds((jj % CSG) * HEADS, HEADS)],
                                ech, sel8[:], start=True, stop=True,
                            )
                            if jj % CSG == CSG - 1:
                                nc.vector.reciprocal_approx_fast(
                                    out=recipT[:, bass.ds(
                                        (jj - CSG + 1) * HEADS, CSG * HEADS)],
                                    in_=cs_ps[:],
                                )
                        nc.vector.tensor_copy(eT[:, bass.ds(off + g0, gw)],
                                              te[:])
                        if pending is not None:
                            emit_ctx(*pending)
                        pending = (off + g0, gw, vT, off)
                    off += SW
                emit_ctx(*pending)

                # ---- context weights: scale rows by 1/rowsum, mask ----
                rowsum = smalls.tile([C, 1], FP32, tag="rowsum")
                nc.vector.tensor_reduce(
                    rowsum[:], rs_acc[:], mybir.AxisListType.X, mybir.AluOpType.add
                )
                rs_rcp = smalls.tile([C, 1], FP32, tag="rs_rcp")
                nc.vector.reciprocal(rs_rcp[:], rowsum[:])
                scaled = smalls.tile([C, C], BF16, tag="scaled")
                nc.vector.tensor_scalar(
                    scaled[:], ctx_ps[:], rs_rcp[:, 0:1], None, mybir.AluOpType.mult
                )
                bd = smalls.tile([C, C], BF16, tag="bd")
                nc.vector.tensor_mul(bd[:], scaled[:], bd8[:])

            # ---- Tail: transposed attended, broadcast normalize, store ----
            with tc.tile_pool(name="psatt", bufs=2, space="PSUM") as ps_att:
                for t in range(NCH // TG):
                    att = ps_att.tile([C, TG * C], FP32, tag="att")
                    for q in range(TG):
                        j = t * TG + q
                        nc.tensor.matmul(
                            att[:, bass.ts(q, C)],
                            exp_nat[:, bass.ts(j, C)],   # stationary (k, n0)
                            bd[:],                        # moving (k, v)
                            start=True, stop=True,
                        )
                    ot = outp.tile([C, TG * C], BF16, tag="ot")
                    a_view = att[:].rearrange(
                        "p (qh i) -> p qh i", i=HC
                    )
                    r_view = recipT[:, bass.ds(t * TG * HEADS, TG * HEADS)] \
                        .unsqueeze(2).broadcast_to([C, TG * HEADS, HC])
                    o_view = ot[:].rearrange(
                        "p (qh i) -> p qh i", i=HC
                    )
                    nc.vector.tensor_tensor(
                        o_view, a_view, r_view, mybir.AluOpType.mult
                    )
                    nc.sync.dma_start(
                        out=out[:, bass.ds(t * TG * C, TG * C)], in_=ot[:]
                    )

    nc.compile()
    return nc


def _get_nc():
    if "nc" not in _cache:
        _cache["nc"] = _build()
    return _cache["nc"]


def _ident_np() -> np.ndarray:
    import ml_dtypes

    return np.eye(C, dtype=np.float32).astype(ml_dtypes.bfloat16)


def _sel8_np() -> np.ndarray:
    import ml_dtypes

    m = np.zeros((C, HEADS), dtype=np.float32)
    for h in range(HEADS):
        m[h * HC:(h + 1) * HC, h] = 1.0
    return m.astype(ml_dtypes.bfloat16)


def _bd8_np() -> np.ndarray:
    import ml_dtypes

    m = np.zeros((C, C), dtype=np.float32)
    for h in range(HEADS):
        m[h * HC:(h + 1) * HC, h * HC:(h + 1) * HC] = 1.0
    return m.astype(ml_dtypes.bfloat16)


def _to_np(a) -> np.ndarray:
    out = np.asarray(a, dtype=np.float32)
    if np.isnan(out).any():
        out = np.asarray(a, dtype=np.float32)
    return out


def _make_in_maps(x1: np.ndarray, x2: np.ndarray) -> list:
    """x1, x2: (B, C, N) float32.  Returns per-core input dicts."""
    import ml_dtypes

    BF = ml_dtypes.bfloat16
    # x1r[p, j*C + c] = x1[b, c, j*128 + p]
    x1r = np.ascontiguousarray(
        x1.reshape(B, C, N // 128, 128).transpose(0, 3, 2, 1)
    ).reshape(B, 128, N).astype(BF)
    x2b = np.ascontiguousarray(x2).astype(BF)
    ident = _ident_np()
    sel8 = _sel8_np()
    bd8 = _bd8_np()
    return [
        {"x1r": x1r[i], "x2": x2b[i], "ident": ident, "sel8": sel8, "bd8": bd8}
        for i in range(NCORES)
    ]


def _unpermute_out(raw: np.ndarray) -> np.ndarray:
    """raw: (C, N) with raw[n0, j*C + v] = att[v, j*128 + n0]."""
    return np.ascontiguousarray(
        raw.reshape(128, NCH, C).transpose(2, 1, 0)
    ).reshape(C, N)


def kernel(x1: np.ndarray, x2: np.ndarray) -> np.ndarray:
    from concourse.bass_utils import run_bass_kernel_spmd

    nc = _get_nc()
    x1 = _to_np(x1).reshape(B, C, N)
    x2 = _to_np(x2).reshape(B, C, N)
    in_maps = _make_in_maps(x1, x2)
    res = run_bass_kernel_spmd(nc, in_maps, core_ids=list(range(NCORES)))
    outs = [
        _unpermute_out(np.asarray(res.results[i]["out"], dtype=np.float32))
        for i in range(NCORES)
    ]
    return np.stack(outs, axis=0).reshape(B, C, H, W)
